# revision 1
# baseline (speedup 1.0000x reference)
"""Fused CE + supervised-contrastive loss on 8 Trainium2 NeuronCores.

Math (reference semantics):
  ce   = -mean_i log_softmax(input)[i, y_i]
  sim  = (X @ X.T) / tau, diag excluded
  lse_i = logsumexp_{k!=i} sim[i,k]
  possum_i = sum_{k!=i, y_k=y_i} sim[i,k] = (x_i . S_{y_i} - ||x_i||^2)/tau
       where S_c = sum_{k: y_k=c} x_k   (class sums -> O(N*C^2), no NxN pass)
  per_i = lse_i - possum_i/n_pos_i  (0 if n_pos_i == 0), n_pos_i = count_{y_i}-1
  loss = (1-lmbd)*ce + lmbd * sum_i per_i

Distribution: each core owns 1024 rows (batch shard) and streams all 8192
columns (full X^T per core).  The only O(N^2) work is the blocked matmul +
exp/accumulate (flash-style logsumexp with a fixed shift), ACT-bound at
~1 elem/cycle/lane (~64us/core).  Class sums S are computed per-core over
the local shard and AllReduce'd (16KB) — triggered first thing so the ncfw
latency (~64us observed) hides completely under the exp phase; all
S-dependent work (G gather, possum) sits at the tail.

Diagonal exclusion: each core's X^T copy is rotated by -1024*core along the
column axis, so row p of block b always has its self-column at local column
b*128+p.  A [128,128] diag(-1e4) accumulate-matmul (start=False) on the PSUM
window kills the diagonal before exp, identically on every core (pure SPMD),
without touching the DVE.

HW quirks handled:
  - tensor_tensor_reduce wedges the device -> use tensor_mul + reduce_sum.
  - ACT Ln is only accurate for inputs in ~[2^-56, 2^64] -> compute ln(se)
    via exponent/mantissa bit-split.
  - fp32 matmul runs at 1/4 rate (fp32r ~1/2 + 2x ldweights) -> bf16
    operands everywhere; error averages out over the 8192-term row sums
    (~1e-5 relative on the final loss).

Outputs per core: [128, 2] per-partition partial sums (SCL, CE).  Host sums
in float64 and combines.
"""

import numpy as np

N, C = 8192, 64
NCORES = 8
RPC = N // NCORES          # rows per core (1024)
P = 128                    # partitions per row-block
NBLK = RPC // P            # 8 row blocks per core
TAU = 0.5
LMBD = 0.5
SHIFT = 100.0              # fixed logsumexp shift
CHUNK = 2048               # columns per PSUM tile (4 banks)
NCHUNK = N // CHUNK        # 4
MM = 512                   # matmul moving free dim (1 PSUM bank)

_CACHE = {}


def _build():
    from contextlib import ExitStack

    import concourse.bass as bass
    import concourse.tile as tile
    from concourse import bacc, mybir

    f32 = mybir.dt.float32
    i32 = mybir.dt.int32
    bf16 = mybir.dt.bfloat16
    AF = mybir.ActivationFunctionType
    ALU = mybir.AluOpType
    AX = mybir.AxisListType

    nc = bacc.Bacc(
        "TRN2",
        target_bir_lowering=False,
        debug=False,
        num_devices=NCORES,
    )

    xt2_d = nc.dram_tensor("xt2", [C, N], bf16, kind="ExternalInput")
    xbt_d = nc.dram_tensor("xbt", [C, RPC], bf16, kind="ExternalInput")
    xaug_d = nc.dram_tensor("xaug", [RPC, C + 1], bf16, kind="ExternalInput")
    ohb_d = nc.dram_tensor("ohb", [RPC, C], bf16, kind="ExternalInput")
    ohbt_d = nc.dram_tensor("ohbt", [C, RPC], bf16, kind="ExternalInput")
    eye_d = nc.dram_tensor("eyeneg", [P, P], bf16, kind="ExternalInput")
    idn_d = nc.dram_tensor("ident", [P, P], bf16, kind="ExternalInput")
    out_d = nc.dram_tensor("out", [P, 2], f32, kind="ExternalOutput")

    def emit(tc, ctx):
        const = ctx.enter_context(tc.tile_pool(name="const", bufs=1))
        dram = ctx.enter_context(tc.tile_pool(name="dram", bufs=1, space="DRAM"))
        psum = ctx.enter_context(tc.tile_pool(name="psum", bufs=2, space="PSUM"))
        escr = ctx.enter_context(tc.tile_pool(name="escr", bufs=2))
        p3s = ctx.enter_context(tc.tile_pool(name="p3s", bufs=4))
        stats = ctx.enter_context(tc.tile_pool(name="stats", bufs=1))

        # ---- input DMAs, ordered so block 0's operands land first ----
        xbt_sb = const.tile([C, RPC], bf16)
        nc.sync.dma_start(xbt_sb[:], xbt_d.ap())
        xt2_sb = const.tile([C, N], bf16)
        nc.sync.dma_start(xt2_sb[:, : N // 4], xt2_d.ap()[:, : N // 4])
        eye_sb = const.tile([P, P], bf16)
        nc.sync.dma_start(eye_sb[:], eye_d.ap())
        idn_sb = const.tile([P, P], bf16)
        nc.sync.dma_start(idn_sb[:], idn_d.ap())
        ohb_sb = const.tile([P, NBLK * C], bf16)
        nc.sync.dma_start(
            ohb_sb[:].rearrange("p (b c) -> p b c", c=C),
            ohb_d.ap().rearrange("(b p) c -> p b c", p=P),
        )
        xaug_sb = const.tile([P, NBLK * (C + 1)], bf16)
        nc.sync.dma_start(
            xaug_sb[:].rearrange("p (b c) -> p b c", c=C + 1),
            xaug_d.ap().rearrange("(b p) c -> p b c", p=P),
        )
        for q in range(1, 4):
            nc.sync.dma_start(
                xt2_sb[:, q * (N // 4) : (q + 1) * (N // 4)],
                xt2_d.ap()[:, q * (N // 4) : (q + 1) * (N // 4)],
            )
        ohbt_sb = const.tile([C, RPC], bf16)
        nc.sync.dma_start(ohbt_sb[:], ohbt_d.ap())

        # ---- persistent tiles ----
        nshift = stats.tile([P, 1], f32)
        nc.vector.memset(nshift[:], -SHIFT)
        esum = stats.tile([P, NBLK * NCHUNK], f32)
        se = stats.tile([P, NBLK], f32)
        nrm = stats.tile([P, NBLK], f32)
        poss = stats.tile([P, NBLK], f32)
        cnt = stats.tile([P, NBLK], f32)
        lgt = stats.tile([P, NBLK], f32)
        cesum = stats.tile([P, NBLK], f32)
        s_loc = stats.tile([C, C + 1], f32)
        s_sb = stats.tile([C, C + 1], bf16)
        res = stats.tile([P, 2], f32)

        # ---- blocked sim + exp accumulate (the O(N^2) part) ----
        def sim_block(b):
            lhs = xbt_sb[:, b * P : (b + 1) * P]
            for n in range(NCHUNK):
                ps = psum.tile([P, CHUNK], f32, tag="ps")
                for k in range(CHUNK // MM):
                    col = n * CHUNK + k * MM
                    nc.tensor.matmul(
                        ps[:, k * MM : (k + 1) * MM],
                        lhsT=lhs,
                        rhs=xt2_sb[:, col : col + MM],
                        start=True,
                        stop=True,
                    )
                if n == 0:
                    # kill self-similarity (local col b*128+p): accumulate
                    # diag(-1e4) on the PE itself, keeping DVE off the path
                    nc.tensor.matmul(
                        ps[:, b * P : (b + 1) * P],
                        lhsT=idn_sb[:],
                        rhs=eye_sb[:],
                        start=False,
                        stop=True,
                        skip_group_check=True,
                    )
                scr = escr.tile([P, CHUNK], f32, tag="scr")
                nc.scalar.activation(
                    scr[:],
                    ps[:],
                    AF.Exp,
                    bias=nshift[:],
                    accum_out=esum[:, b * NCHUNK + n : b * NCHUNK + n + 1],
                )
            # per-block stats that don't need the AllReduced S
            xb = xaug_sb[:, b * (C + 1) : b * (C + 1) + C]
            oh = ohb_sb[:, b * C : (b + 1) * C]
            t0 = p3s.tile([P, C], f32, tag="p3")
            nc.vector.tensor_mul(t0[:], xb, xb)
            nc.vector.reduce_sum(nrm[:, b : b + 1], t0[:], axis=AX.X)
            t2 = p3s.tile([P, C], f32, tag="p3")
            nc.vector.tensor_mul(t2[:], xb, oh)
            nc.vector.reduce_sum(lgt[:, b : b + 1], t2[:], axis=AX.X)
            nc.vector.reduce_sum(
                se[:, b : b + 1],
                esum[:, b * NCHUNK : (b + 1) * NCHUNK],
                axis=AX.X,
            )

        # block 0 leads so PE work starts on the earliest-landing DMAs
        sim_block(0)

        # ---- class sums S_r + AllReduce: emitted right after block 0 so
        # the collective's ncfw latency (~64us observed) hides under the
        # remaining exp phase ----
        s_ps = psum.tile([P, CHUNK], f32, tag="ps")
        s_view = s_ps[:C, : C + 1]
        for b in range(NBLK):
            nc.tensor.matmul(
                s_view,
                lhsT=ohb_sb[:, b * C : (b + 1) * C],
                rhs=xaug_sb[:, b * (C + 1) : (b + 1) * (C + 1)],
                start=(b == 0),
                stop=(b == NBLK - 1),
            )
        nc.vector.tensor_copy(s_loc[:], s_view)
        s_in = dram.tile([C, C + 1], f32)
        s_out = dram.tile([C, C + 1], f32)
        nc.sync.dma_start(s_in[:], s_loc[:])
        nc.gpsimd.collective_compute(
            "AllReduce",
            mybir.AluOpType.add,
            replica_groups=[list(range(NCORES))],
            ins=[s_in.opt()],
            outs=[s_out.opt()],
        )
        nc.gpsimd.dma_start(s_sb[:], s_out[:])  # casts f32 -> bf16

        for b in range(1, NBLK):
            sim_block(b)

        # CE denominators: one batched exp over all 8 blocks (cheaper on the
        # bottleneck ACT than 8 small accum_out calls), reduced by the
        # otherwise-idle DVE
        cescr = p3s.tile([P, NBLK * (C + 1)], f32, tag="cescr")
        nc.scalar.activation(cescr[:], xaug_sb[:], AF.Exp)
        for b in range(NBLK):
            nc.vector.reduce_sum(
                cesum[:, b : b + 1],
                cescr[:, b * (C + 1) : b * (C + 1) + C],
                axis=AX.X,
            )

        # ---- G = onehot_b @ S for all blocks into one PSUM tile ----
        g_all = psum.tile([P, CHUNK], f32, tag="ps")
        GS = 256  # per-block column stride keeps each output inside a bank
        for b in range(NBLK):
            nc.tensor.matmul(
                g_all[:, b * GS : b * GS + C + 1],
                lhsT=ohbt_sb[:, b * P : (b + 1) * P],
                rhs=s_sb[:],
                start=True,
                stop=True,
            )
        for b in range(NBLK):
            xb = xaug_sb[:, b * (C + 1) : b * (C + 1) + C]
            t1 = p3s.tile([P, C], f32, tag="p3")
            nc.vector.tensor_mul(t1[:], xb, g_all[:, b * GS : b * GS + C])
            nc.vector.reduce_sum(poss[:, b : b + 1], t1[:], axis=AX.X)
            nc.vector.tensor_copy(
                cnt[:, b : b + 1], g_all[:, b * GS + C : b * GS + C + 1]
            )

        # ---- final per-row assembly ----
        fin = stats
        # robust ln(se): se = m * 2^e, ln = Ln(m) + e*ln2 (HW Ln is only
        # accurate in ~[2^-56, 2^64]; se spans e^-60..e^+26)
        sec = fin.tile([P, NBLK], f32)
        nc.vector.tensor_scalar_max(sec[:], se[:], 1e-37)
        bits = sec[:].bitcast(i32)
        exi = fin.tile([P, NBLK], i32)
        nc.vector.tensor_scalar(
            out=exi[:], in0=bits, scalar1=23, scalar2=None,
            op0=ALU.arith_shift_right,
        )
        exf = fin.tile([P, NBLK], f32)
        nc.vector.tensor_copy(exf[:], exi[:])
        mbits = fin.tile([P, NBLK], i32)
        nc.vector.tensor_scalar(
            out=mbits[:], in0=bits, scalar1=0x007FFFFF, scalar2=0x3F800000,
            op0=ALU.bitwise_and, op1=ALU.bitwise_or,
        )
        lnm = fin.tile([P, NBLK], f32)
        nc.scalar.activation(lnm[:], mbits[:].bitcast(f32), AF.Ln)
        eln2 = fin.tile([P, NBLK], f32)
        nc.vector.tensor_scalar(
            out=eln2[:], in0=exf[:], scalar1=-127.0,
            scalar2=float(np.log(2.0)), op0=ALU.add, op1=ALU.mult,
        )
        lnse = fin.tile([P, NBLK], f32)
        nc.vector.tensor_add(lnse[:], lnm[:], eln2[:])

        nposc = fin.tile([P, NBLK], f32)
        nc.vector.tensor_scalar(
            out=nposc[:], in0=cnt[:], scalar1=-1.0, scalar2=1.0,
            op0=ALU.add, op1=ALU.max,
        )
        mask = fin.tile([P, NBLK], f32)
        nc.vector.tensor_scalar(
            out=mask[:], in0=cnt[:], scalar1=-1.0, scalar2=1.0,
            op0=ALU.add, op1=ALU.min,
        )
        rc = fin.tile([P, NBLK], f32)
        nc.vector.reciprocal(rc[:], nposc[:])

        pd = fin.tile([P, NBLK], f32)
        nc.vector.tensor_sub(pd[:], poss[:], nrm[:])
        pt = fin.tile([P, NBLK], f32)
        nc.vector.scalar_tensor_tensor(
            out=pt[:], in0=pd[:], scalar=1.0 / TAU, in1=rc[:],
            op0=ALU.mult, op1=ALU.mult,
        )
        peri = fin.tile([P, NBLK], f32)
        nc.vector.scalar_tensor_tensor(
            out=peri[:], in0=lnse[:], scalar=SHIFT, in1=pt[:],
            op0=ALU.add, op1=ALU.subtract,
        )
        perim = fin.tile([P, NBLK], f32)
        nc.vector.tensor_mul(perim[:], peri[:], mask[:])

        lnce = fin.tile([P, NBLK], f32)
        nc.scalar.activation(lnce[:], cesum[:], AF.Ln)
        cec = fin.tile([P, NBLK], f32)
        nc.vector.tensor_sub(cec[:], lnce[:], lgt[:])

        nc.vector.reduce_sum(res[:, 0:1], perim[:], axis=AX.X)
        nc.vector.reduce_sum(res[:, 1:2], cec[:], axis=AX.X)
        nc.sync.dma_start(out_d.ap(), res[:])

    with tile.TileContext(nc) as tc, ExitStack() as ctx:
        emit(tc, ctx)

    nc.compile()
    return nc


def _get_nc(**kw):
    key = repr(sorted(kw.items()))
    if key not in _CACHE:
        _CACHE[key] = _build(**kw)
    return _CACHE[key]


def _make_in_maps(X, y):
    import ml_dtypes

    bf = ml_dtypes.bfloat16
    X = np.ascontiguousarray(np.asarray(X, dtype=np.float32))
    y = np.asarray(y).astype(np.int64).ravel()
    assert X.shape == (N, C) and y.shape == (N,)

    oh = (y[:, None] == np.arange(C)[None, :]).astype(bf)
    xt2 = np.ascontiguousarray((X.T / np.float32(TAU)).astype(bf))
    eyeneg = (np.eye(P) * -1e4).astype(bf)
    ident = np.eye(P).astype(bf)

    in_maps = []
    for r in range(NCORES):
        rows = slice(r * RPC, (r + 1) * RPC)
        xb = X[rows]
        in_maps.append(
            {
                "xt2": np.ascontiguousarray(np.roll(xt2, -r * RPC, axis=1)),
                "xbt": np.ascontiguousarray(xb.T.astype(bf)),
                "xaug": np.ascontiguousarray(
                    np.concatenate(
                        [xb, np.ones((RPC, 1), np.float32)], axis=1
                    ).astype(bf)
                ),
                "ohb": np.ascontiguousarray(oh[rows]),
                "ohbt": np.ascontiguousarray(oh[rows].T),
                "eyeneg": eyeneg,
                "ident": ident,
            }
        )
    return in_maps


def run(input, target, trace=False, **build_kw):
    """Run the device kernel; returns (loss_scalar, BassKernelResults)."""
    from concourse.bass_utils import run_bass_kernel_spmd

    nc = _get_nc(**build_kw)
    in_maps = _make_in_maps(input, target)
    res = run_bass_kernel_spmd(
        nc, in_maps, core_ids=list(range(NCORES)), trace=trace
    )
    sc = 0.0
    ce = 0.0
    for core_out in res.results:
        o = core_out["out"].astype(np.float64)
        sc += o[:, 0].sum()
        ce += o[:, 1].sum()
    loss = (1.0 - LMBD) * (ce / N) + LMBD * sc
    return np.array(loss, dtype=np.float32), res


def kernel(input, target):
    loss, _ = run(input, target, trace=False)
    return loss



# revision 3
# speedup vs baseline: 1.3766x; 1.3766x over previous
"""Fused CE + supervised-contrastive loss on 8 Trainium2 NeuronCores.

Math (reference semantics):
  ce   = -mean_i log_softmax(input)[i, y_i]
  sim  = (X @ X.T) / tau, diag excluded
  lse_i = logsumexp_{k!=i} sim[i,k]
  possum_i = (x_i . S_{y_i} - ||x_i||^2)/tau,  S_c = sum_{y_k=c} x_k
  per_i = lse_i - possum_i/n_pos_i (0 if n_pos_i == 0)
  loss = (1-lmbd)*ce + lmbd * sum_i per_i

Strategy (v2, symmetric-pair + tile-parallel PE):
  The only O(N^2) work is the exp row-sums se_i = sum_k exp(sim_ik - SHIFT).
  Each core owns 8 contiguous 128-row blocks. Row-block I processes a
  wrap-around window of W=40 column blocks j=0..39:
    - j=0: diagonal block, self-sim killed in PSUM by two 64-contraction
      "eye" accumulate-matmuls (quadrant trick keeps 64x128 tile mode).
    - j=1..24 ("symmetric"): exp'd once; row-sums via ACT/DVE accumulate,
      column-sums (the mirrored pairs' row-sums) via ones-at-s packed
      matmuls: lhsT has its ones in column s, so PSUM row s accumulates the
      strip's colsum while other rows get +0 -- 48 strips pack into one
      [128,512] PSUM tile per contraction half, drained once at the end.
    - j=25..39 ("both-directions"): distances 25..32 are computed from both
      sides (j=d and j=64-d), so no colsum is needed.
  Every matmul is 64-contraction in 64x128 tile mode: T0 instructions use
  SBUF partitions 0-63, T8 partitions 64-127, and the two tiles execute
  CONCURRENTLY (measured 2x; mode switches wedge the device, so the kernel
  never leaves 64x128 mode).  X^T/sqrt(tau) is duplicated into both
  partition halves so any chunk can go to either tile.

  exp work is split ACT (exact spline exp, ~1.2ns/col incl. overhead) vs
  DVE (Schraudolph bf16 fast-exp: clamp+scale, +bias -> int16 bitcast,
  reduce; ~2.9ns/col) to run both engines in parallel.  DVE terms carry a
  calibrated ~0.2% bias -> ~1e-5 on the final loss (tolerance 2e-2).

  Host does O(N*C) prep/post: class sums S -> G=S[y] upload, and the final
  per-row assembly (ln(se), possum, CE combine) in float64.
"""

import numpy as np

N, C = 8192, 64
NCORES = 8
RPC = N // NCORES          # 1024 rows per core
P = 128
NBLK = RPC // P            # 8 row-blocks per core
TAU = 0.5
LMBD = 0.5
SHIFT = 100.0

SYMJ = 24                  # colsum distances 1..SYMJ
WINB = 64 - SYMJ           # 40 window blocks (j = 0..39)
WCOLS = WINB * P           # 5120 window cols
CS_COLS = SYMJ * P         # 3072 colsum cols -> 6 strips of 512
STRIPS = CS_COLS // 512    # 6
XTW = (NBLK - 1) * P + WCOLS  # 6016... (7*128 + 5120) = 6016

# chunking of the 5120-col window: [T0 1536][T8 1536][T0 1536][T8 512]
C0, C1, C2, C3 = 1536, 1536, 1536, 512
ACT_PRE = 640              # ACT takes c1[0:640], DVE c1[640:1536] + c3

# Schraudolph bf16 fast-exp constants (calibrated)
A16 = 128.0 / float(np.log(2.0))
SIGMA = 3.25
B16P = 16256.0 - SIGMA - SHIFT * A16
CLAMP_LO = SHIFT - 88.0    # 12.0

_CACHE = {}


def _build():
    from contextlib import ExitStack

    import concourse.bass as bass
    import concourse.tile as tile
    from concourse import bacc, mybir

    f32 = mybir.dt.float32
    i16 = mybir.dt.int16
    bf16 = mybir.dt.bfloat16
    AF = mybir.ActivationFunctionType
    ALU = mybir.AluOpType
    AX = mybir.AxisListType

    nc = bacc.Bacc(
        "TRN2",
        target_bir_lowering=False,
        debug=False,
        num_devices=NCORES,
    )

    xt2_d = nc.dram_tensor("xt2", [P, XTW], bf16, kind="ExternalInput")
    xrows_d = nc.dram_tensor("xrows", [P, NBLK * C], bf16, kind="ExternalInput")
    g_d = nc.dram_tensor("g", [P, NBLK * C], bf16, kind="ExternalInput")
    zwin_d = nc.dram_tensor("zwin", [P, 184], bf16, kind="ExternalInput")
    dkit_d = nc.dram_tensor("dkit", [64, 4 * P], bf16, kind="ExternalInput")
    stats_d = nc.dram_tensor("stats", [P, 56], f32, kind="ExternalOutput")
    csa_d = nc.dram_tensor("csa", [P, 512], f32, kind="ExternalOutput")
    csb_d = nc.dram_tensor("csb", [P, 512], f32, kind="ExternalOutput")

    def emit(tc, ctx):
        const = ctx.enter_context(tc.tile_pool(name="const", bufs=1))
        psum = ctx.enter_context(tc.tile_pool(name="psum", bufs=1, space="PSUM"))
        epool = ctx.enter_context(tc.tile_pool(name="epool", bufs=2))
        m1p = ctx.enter_context(tc.tile_pool(name="m1p", bufs=2))
        pmp = ctx.enter_context(tc.tile_pool(name="pmp", bufs=2))
        stats = ctx.enter_context(tc.tile_pool(name="stats", bufs=1))

        # ---- input DMAs, ordered so window 0's operands land first ----
        zwin_sb = const.tile([P, 184], bf16)
        nc.sync.dma_start(zwin_sb[:], zwin_d.ap())
        dkit_sb = const.tile([64, 4 * P], bf16)
        nc.sync.dma_start(dkit_sb[:], dkit_d.ap())
        xt2_sb = const.tile([P, XTW], bf16)
        nc.sync.dma_start(xt2_sb[:, 0:2048], xt2_d.ap()[:, 0:2048])
        nc.sync.dma_start(xt2_sb[:, 2048:4096], xt2_d.ap()[:, 2048:4096])
        nc.sync.dma_start(xt2_sb[:, 4096:XTW], xt2_d.ap()[:, 4096:XTW])
        xrows_sb = const.tile([P, NBLK * C], bf16)
        nc.sync.dma_start(xrows_sb[:], xrows_d.ap())
        g_sb = const.tile([P, NBLK * C], bf16)
        nc.sync.dma_start(g_sb[:], g_d.ap())

        # ---- persistent tiles ----
        nshift = stats.tile([P, 1], f32)
        nc.vector.memset(nshift[:], -SHIFT)
        st = stats.tile([P, 56], f32)

        pT0 = psum.tile([P, C0], f32, name="pT0")
        pT8 = psum.tile([P, C1], f32, name="pT8")
        csA = psum.tile([P, 512], f32, name="csA")
        csB = psum.tile([P, 512], f32, name="csB")

        dk = dkit_sb
        for w in range(NBLK):
            base = w * P
            lhs0 = xt2_sb[0:64, base:base + P]
            lhs8 = xt2_sb[64:128, base:base + P]
            E = epool.tile([P, WCOLS], bf16, tag="E", name="E")
            Ei = E[:].bitcast(i16)

            # c0 (T0): window cols [0:1536)
            for k in range(3):
                a = k * 512
                nc.tensor.matmul(
                    pT0[:, a:a + 512], lhsT=lhs0,
                    rhs=xt2_sb[0:64, base + a:base + a + 512],
                    start=True, stop=True,
                )
            # diagonal kill on cols [0:128) (two 64-contraction quadrants)
            nc.tensor.matmul(
                pT0[:, 0:P], lhsT=dk[:, 0:P], rhs=dk[:, P:2 * P],
                start=False, stop=True, skip_group_check=True,
            )
            nc.tensor.matmul(
                pT0[:, 0:P], lhsT=dk[:, 2 * P:3 * P], rhs=dk[:, 3 * P:4 * P],
                start=False, stop=True, skip_group_check=True,
            )
            # c1 (T8): window cols [1536:3072)
            for k in range(3):
                a = k * 512
                nc.tensor.matmul(
                    pT8[:, a:a + 512], lhsT=lhs8,
                    rhs=xt2_sb[64:128, base + C0 + a:base + C0 + a + 512],
                    start=True, stop=True,
                )
            # ACT exp c0 -> E[0:1536), rowsum to st col w*5+0
            nc.scalar.activation(
                E[:, 0:C0], pT0[:], AF.Exp, bias=nshift[:],
                accum_out=st[:, w * 5:w * 5 + 1],
            )
            # ACT exp c1 prefix [1536:2176)
            nc.scalar.activation(
                E[:, C0:C0 + ACT_PRE], pT8[:, 0:ACT_PRE], AF.Exp,
                bias=nshift[:], accum_out=st[:, w * 5 + 1:w * 5 + 2],
            )
            # DVE fast-exp c1 suffix [2176:3072)
            m1a = m1p.tile([P, C1 - ACT_PRE], f32, tag="m1a", name="m1a")
            nc.vector.tensor_scalar(
                out=m1a[:], in0=pT8[:, ACT_PRE:C1],
                scalar1=CLAMP_LO, scalar2=A16, op0=ALU.max, op1=ALU.mult,
            )
            nc.vector.tensor_scalar(
                out=Ei[:, C0 + ACT_PRE:C0 + C1], in0=m1a[:],
                scalar1=B16P, scalar2=None, op0=ALU.add,
            )
            nc.vector.reduce_sum(
                st[:, w * 5 + 2:w * 5 + 3], E[:, C0 + ACT_PRE:C0 + C1],
                axis=AX.X,
            )
            # c2 (T0): window cols [3072:4608)
            for k in range(3):
                a = k * 512
                nc.tensor.matmul(
                    pT0[:, a:a + 512], lhsT=lhs0,
                    rhs=xt2_sb[0:64, base + C0 + C1 + a:base + C0 + C1 + a + 512],
                    start=True, stop=True,
                )
            nc.scalar.activation(
                E[:, C0 + C1:C0 + C1 + C2], pT0[:], AF.Exp, bias=nshift[:],
                accum_out=st[:, w * 5 + 3:w * 5 + 4],
            )
            # c3 (T8): window cols [4608:5120)
            nc.tensor.matmul(
                pT8[:, 0:C3], lhsT=lhs8,
                rhs=xt2_sb[64:128, base + C0 + C1 + C2:base + WCOLS],
                start=True, stop=True,
            )
            m1b = m1p.tile([P, C3], f32, tag="m1b", name="m1b")
            nc.vector.tensor_scalar(
                out=m1b[:], in0=pT8[:, 0:C3],
                scalar1=CLAMP_LO, scalar2=A16, op0=ALU.max, op1=ALU.mult,
            )
            nc.vector.tensor_scalar(
                out=Ei[:, C0 + C1 + C2:WCOLS], in0=m1b[:],
                scalar1=B16P, scalar2=None, op0=ALU.add,
            )
            nc.vector.reduce_sum(
                st[:, w * 5 + 4:w * 5 + 5], E[:, C0 + C1 + C2:WCOLS],
                axis=AX.X,
            )
            # colsum strips t=0..5 over E cols [128+512t : 640+512t)
            for t in range(STRIPS):
                s = STRIPS * w + t
                a = P + 512 * t
                nc.tensor.matmul(
                    csA[:], lhsT=zwin_sb[0:64, 47 - s:47 - s + P],
                    rhs=E[0:64, a:a + 512],
                    start=(s == 0), stop=(s == STRIPS * NBLK - 1),
                    skip_group_check=True,
                )
                nc.tensor.matmul(
                    csB[:], lhsT=zwin_sb[64:128, 47 - s:47 - s + P],
                    rhs=E[64:128, a:a + 512],
                    start=(s == 0), stop=(s == STRIPS * NBLK - 1),
                    skip_group_check=True,
                )
            # possum dot: x_i . G_i for this block
            pm = pmp.tile([P, C], f32, tag="pm", name="pm")
            nc.vector.tensor_mul(
                pm[:], xrows_sb[:, w * C:(w + 1) * C],
                g_sb[:, w * C:(w + 1) * C],
            )
            nc.vector.reduce_sum(st[:, 40 + w:41 + w], pm[:], axis=AX.X)

        # CE denominators: one exp over all logits, then per-block reduces
        cescr = epool.tile([P, NBLK * C], f32, tag="cescr", name="cescr")
        nc.scalar.activation(cescr[:], xrows_sb[:], AF.Exp)
        for w in range(NBLK):
            nc.vector.reduce_sum(
                st[:, 48 + w:49 + w], cescr[:, w * C:(w + 1) * C], axis=AX.X,
            )

        nc.sync.dma_start(stats_d.ap(), st[:])
        csa_sb = stats.tile([P, 512], f32)
        nc.vector.tensor_copy(csa_sb[:], csA[:])
        nc.sync.dma_start(csa_d.ap(), csa_sb[:])
        csb_sb = stats.tile([P, 512], f32)
        nc.vector.tensor_copy(csb_sb[:], csB[:])
        nc.sync.dma_start(csb_d.ap(), csb_sb[:])

    with tile.TileContext(nc) as tc, ExitStack() as ctx:
        emit(tc, ctx)

    nc.compile()
    return nc


def _get_nc(**kw):
    key = repr(sorted(kw.items()))
    if key not in _CACHE:
        _CACHE[key] = _build(**kw)
    return _CACHE[key]


def _prep(X, y):
    import ml_dtypes

    bf = ml_dtypes.bfloat16
    X = np.ascontiguousarray(np.asarray(X, dtype=np.float32))
    y = np.asarray(y).astype(np.int64).ravel()
    assert X.shape == (N, C) and y.shape == (N,)

    xs = (X.T / np.float32(np.sqrt(TAU))).astype(np.float32)  # [C, N]

    # class sums and per-row gathers (host, O(N*C))
    S = np.zeros((C, C), np.float64)
    np.add.at(S, y, X.astype(np.float64))
    G = S[y].astype(np.float32)                                # [N, C]
    cnt = np.bincount(y, minlength=C)[y].astype(np.float64)    # incl self
    nrm = (X.astype(np.float64) ** 2).sum(1)
    logit = X[np.arange(N), y].astype(np.float64)

    # ones-at-s window for colsum packing: col 47 is all-ones
    zwin = np.zeros((P, 184), np.float32)
    zwin[:, 47] = 1.0

    # diag-kill quadrants (64-contraction)
    dkit = np.zeros((64, 4 * P), np.float32)
    e64 = np.eye(64, dtype=np.float32)
    dkit[:, 0:64] = e64                  # lhsT TL: col p = e_p (p<64)
    dkit[:, P:P + 64] = -1e4 * e64       # rhs TL
    dkit[:, 2 * P + 64:3 * P] = e64      # lhsT BR: col p = e_{p-64} (p>=64)
    dkit[:, 3 * P + 64:4 * P] = -1e4 * e64
    zwin_bf = zwin.astype(bf)
    dkit_bf = dkit.astype(bf)

    in_maps = []
    for r in range(NCORES):
        rows = slice(r * RPC, (r + 1) * RPC)
        xl = np.roll(xs, -r * RPC, axis=1)[:, :XTW]            # [64, XTW]
        xt2 = np.concatenate([xl, xl], axis=0).astype(bf)      # [128, XTW]
        xr = X[rows].reshape(NBLK, P, C).transpose(1, 0, 2).reshape(P, NBLK * C)
        gr = G[rows].reshape(NBLK, P, C).transpose(1, 0, 2).reshape(P, NBLK * C)
        in_maps.append({
            "xt2": np.ascontiguousarray(xt2),
            "xrows": np.ascontiguousarray(xr.astype(bf)),
            "g": np.ascontiguousarray(gr.astype(bf)),
            "zwin": zwin_bf,
            "dkit": dkit_bf,
        })
    host = {"cnt": cnt, "nrm": nrm, "logit": logit}
    return in_maps, host


def _combine(results, host):
    se = np.zeros(N, np.float64)
    p_dot = np.zeros(N, np.float64)
    cesum = np.zeros(N, np.float64)
    idx = np.arange(512)
    for r, core_out in enumerate(results):
        st = core_out["stats"].astype(np.float64)
        cs = (core_out["csa"].astype(np.float64)
              + core_out["csb"].astype(np.float64))
        for w in range(NBLK):
            rows = slice((8 * r + w) * P, (8 * r + w) * P + P)
            se[rows] += st[:, w * 5:w * 5 + 5].sum(axis=1)
            p_dot[rows] = st[:, 40 + w]
            cesum[rows] = st[:, 48 + w]
            for t in range(STRIPS):
                gbase = r * RPC + w * P + P + 512 * t
                gi = (gbase + idx) % N
                se[gi] += cs[STRIPS * w + t]
    lse = np.log(se) + SHIFT
    possum = (p_dot - host["nrm"]) / TAU
    npos = host["cnt"] - 1.0
    per_i = np.where(
        npos > 0, lse - possum / np.maximum(npos, 1.0), 0.0
    )
    sc = per_i.sum()
    ce = (np.log(cesum) - host["logit"]).mean()
    return np.float32((1.0 - LMBD) * ce + LMBD * sc)


def run(input, target, trace=False, **build_kw):
    """Run the device kernel; returns (loss_scalar, BassKernelResults)."""
    from concourse.bass_utils import run_bass_kernel_spmd

    nc = _get_nc(**build_kw)
    in_maps, host = _prep(input, target)
    res = run_bass_kernel_spmd(
        nc, in_maps, core_ids=list(range(NCORES)), trace=trace
    )
    loss = _combine(res.results, host)
    return loss, res


def kernel(input, target):
    loss, _ = run(input, target, trace=False)
    return loss


# revision 8
# speedup vs baseline: 1.4406x; 1.0465x over previous
"""Fused CE + supervised-contrastive loss on 8 Trainium2 NeuronCores.

Math (reference semantics):
  ce   = -mean_i log_softmax(input)[i, y_i]
  sim  = (X @ X.T) / tau, diag excluded
  lse_i = logsumexp_{k!=i} sim[i,k]
  possum_i = (x_i . S_{y_i} - ||x_i||^2)/tau,  S_c = sum_{y_k=c} x_k
  per_i = lse_i - possum_i/n_pos_i (0 if n_pos_i == 0)
  loss = (1-lmbd)*ce + lmbd * sum_i per_i

Strategy (v2, symmetric-pair + tile-parallel PE):
  The only O(N^2) work is the exp row-sums se_i = sum_k exp(sim_ik - SHIFT).
  Each core owns 8 contiguous 128-row blocks. Row-block I processes a
  wrap-around window of W=40 column blocks j=0..39:
    - j=0: diagonal block, self-sim killed in PSUM by two 64-contraction
      "eye" accumulate-matmuls (quadrant trick keeps 64x128 tile mode).
    - j=1..24 ("symmetric"): exp'd once; row-sums via ACT/DVE accumulate,
      column-sums (the mirrored pairs' row-sums) via ones-at-s packed
      matmuls: lhsT has its ones in column s, so PSUM row s accumulates the
      strip's colsum while other rows get +0 -- 48 strips pack into one
      [128,512] PSUM tile per contraction half, drained once at the end.
    - j=25..39 ("both-directions"): distances 25..32 are computed from both
      sides (j=d and j=64-d), so no colsum is needed.
  Every matmul is 64-contraction in 64x128 tile mode: T0 instructions use
  SBUF partitions 0-63, T8 partitions 64-127, and the two tiles execute
  CONCURRENTLY (measured 2x; mode switches wedge the device, so the kernel
  never leaves 64x128 mode).  X^T/sqrt(tau) is duplicated into both
  partition halves so any chunk can go to either tile.

  exp work is split ACT (exact spline exp, ~1.2ns/col incl. overhead) vs
  DVE (Schraudolph bf16 fast-exp: clamp+scale, +bias -> int16 bitcast,
  reduce; ~2.9ns/col) to run both engines in parallel.  DVE terms carry a
  calibrated ~0.2% bias -> ~1e-5 on the final loss (tolerance 2e-2).

  Host does O(N*C) prep/post: class sums S -> G=S[y] upload, and the final
  per-row assembly (ln(se), possum, CE combine) in float64.
"""

import numpy as np

N, C = 8192, 64
NCORES = 8
RPC = N // NCORES          # 1024 rows per core
P = 128
NBLK = RPC // P            # 8 row-blocks per core
TAU = 0.5
LMBD = 0.5
SHIFT = 100.0

SYMJ = 24                  # colsum distances 1..SYMJ
WINB = 64 - SYMJ           # 40 window blocks (j = 0..39)
WCOLS = WINB * P           # 5120 window cols
CS_COLS = SYMJ * P         # 3072 colsum cols -> 6 strips of 512
STRIPS = CS_COLS // 512    # 6
XTW = (NBLK - 1) * P + WCOLS  # 6016... (7*128 + 5120) = 6016

# chunking of the 5120-col window: [T0 1536][T8 1536][T0 1536][T8 512]
C0, C1, C2, C3 = 1536, 1536, 1536, 512
ACT_PRE = 640              # ACT takes c1[0:640], DVE c1[640:1536] + c3

# Schraudolph bf16 fast-exp constants (calibrated)
A16 = 128.0 / float(np.log(2.0))
SIGMA = 3.25
B16P = 16256.0 - SIGMA - SHIFT * A16
CLAMP_LO = SHIFT - 88.0    # 12.0

_CACHE = {}


def _build():
    from contextlib import ExitStack

    import concourse.bass as bass
    import concourse.tile as tile
    from concourse import bacc, mybir

    f32 = mybir.dt.float32
    i16 = mybir.dt.int16
    bf16 = mybir.dt.bfloat16
    AF = mybir.ActivationFunctionType
    ALU = mybir.AluOpType
    AX = mybir.AxisListType

    nc = bacc.Bacc(
        "TRN2",
        target_bir_lowering=False,
        debug=False,
        num_devices=NCORES,
    )

    xt2_d = nc.dram_tensor("xt2", [P, XTW], bf16, kind="ExternalInput")
    xrows_d = nc.dram_tensor("xrows", [P, NBLK * C], bf16, kind="ExternalInput")
    g_d = nc.dram_tensor("g", [P, NBLK * C], bf16, kind="ExternalInput")
    zwin_d = nc.dram_tensor("zwin", [P, 184], bf16, kind="ExternalInput")
    dkit_d = nc.dram_tensor("dkit", [64, 4 * P], bf16, kind="ExternalInput")
    stats_d = nc.dram_tensor("stats", [P, 56], f32, kind="ExternalOutput")
    csa_d = nc.dram_tensor("csa", [P, 512], f32, kind="ExternalOutput")
    csb_d = nc.dram_tensor("csb", [P, 512], f32, kind="ExternalOutput")

    def emit(tc, ctx):
        const = ctx.enter_context(tc.tile_pool(name="const", bufs=1))
        psum = ctx.enter_context(tc.tile_pool(name="psum", bufs=1, space="PSUM"))
        epool = ctx.enter_context(tc.tile_pool(name="epool", bufs=2))
        m1p = ctx.enter_context(tc.tile_pool(name="m1p", bufs=2))
        pmp = ctx.enter_context(tc.tile_pool(name="pmp", bufs=2))
        stats = ctx.enter_context(tc.tile_pool(name="stats", bufs=1))

        # ---- input DMAs, ordered so window 0's operands land first ----
        zwin_sb = const.tile([P, 184], bf16)
        nc.sync.dma_start(zwin_sb[:], zwin_d.ap())
        dkit_sb = const.tile([64, 4 * P], bf16)
        nc.sync.dma_start(dkit_sb[:], dkit_d.ap())
        xt2_sb = const.tile([P, XTW], bf16)
        nc.sync.dma_start(xt2_sb[:, 0:1664], xt2_d.ap()[:, 0:1664])
        nc.sync.dma_start(xt2_sb[:, 1664:4096], xt2_d.ap()[:, 1664:4096])
        nc.sync.dma_start(xt2_sb[:, 4096:XTW], xt2_d.ap()[:, 4096:XTW])
        xrows_sb = const.tile([P, NBLK * C], bf16)
        nc.sync.dma_start(xrows_sb[:], xrows_d.ap())
        g_sb = const.tile([P, NBLK * C], bf16)
        nc.sync.dma_start(g_sb[:], g_d.ap())

        # ---- persistent tiles ----
        nshift = stats.tile([P, 1], f32)
        nc.vector.memset(nshift[:], -SHIFT)
        st = stats.tile([P, 56], f32)

        pT0 = psum.tile([P, C0], f32, name="pT0")
        pT8 = psum.tile([P, C1], f32, name="pT8")
        csA = psum.tile([P, 512], f32, name="csA")
        csB = psum.tile([P, 512], f32, name="csB")

        dk = dkit_sb

        def colsum_strips(w, E):
            # strips for window w read E regions produced ~a window ago, so
            # emitting them inside window w+1 keeps the in-order PE queue
            # from stalling on the freshest exp region
            for t in range(STRIPS):
                s = STRIPS * w + t
                a = P + 512 * t
                nc.tensor.matmul(
                    csA[:], lhsT=zwin_sb[0:64, 47 - s:47 - s + P],
                    rhs=E[0:64, a:a + 512],
                    start=(s == 0), stop=(s == STRIPS * NBLK - 1),
                    skip_group_check=True,
                )
                nc.tensor.matmul(
                    csB[:], lhsT=zwin_sb[64:128, 47 - s:47 - s + P],
                    rhs=E[64:128, a:a + 512],
                    start=(s == 0), stop=(s == STRIPS * NBLK - 1),
                    skip_group_check=True,
                )

        prevE = None
        for w in range(NBLK):
            base = w * P
            lhs0 = xt2_sb[0:64, base:base + P]
            lhs8 = xt2_sb[64:128, base:base + P]
            E = epool.tile([P, WCOLS], bf16, tag="E", name="E")
            Ei = E[:].bitcast(i16)

            # c0 (T0): window cols [0:1536)
            for k in range(3):
                a = k * 512
                nc.tensor.matmul(
                    pT0[:, a:a + 512], lhsT=lhs0,
                    rhs=xt2_sb[0:64, base + a:base + a + 512],
                    start=True, stop=True,
                )
            # diagonal kill on cols [0:128) (two 64-contraction quadrants)
            nc.tensor.matmul(
                pT0[:, 0:P], lhsT=dk[:, 0:P], rhs=dk[:, P:2 * P],
                start=False, stop=True, skip_group_check=True,
            )
            nc.tensor.matmul(
                pT0[:, 0:P], lhsT=dk[:, 2 * P:3 * P], rhs=dk[:, 3 * P:4 * P],
                start=False, stop=True, skip_group_check=True,
            )
            # c1 (T8): window cols [1536:3072)
            for k in range(3):
                a = k * 512
                nc.tensor.matmul(
                    pT8[:, a:a + 512], lhsT=lhs8,
                    rhs=xt2_sb[64:128, base + C0 + a:base + C0 + a + 512],
                    start=True, stop=True,
                )
            # previous window's colsum strips (data long ready: no PE stall)
            if prevE is not None:
                colsum_strips(w - 1, prevE)
            # ACT exp c0 -> E[0:1536), rowsum to st col w*5+0
            nc.scalar.activation(
                E[:, 0:C0], pT0[:], AF.Exp, bias=nshift[:],
                accum_out=st[:, w * 5:w * 5 + 1],
            )
            # ACT exp c1 prefix [1536:2176)
            nc.scalar.activation(
                E[:, C0:C0 + ACT_PRE], pT8[:, 0:ACT_PRE], AF.Exp,
                bias=nshift[:], accum_out=st[:, w * 5 + 1:w * 5 + 2],
            )
            # DVE fast-exp c1 suffix [2176:3072)
            m1a = m1p.tile([P, C1 - ACT_PRE], f32, tag="m1a", name="m1a")
            nc.vector.tensor_scalar(
                out=m1a[:], in0=pT8[:, ACT_PRE:C1],
                scalar1=CLAMP_LO, scalar2=A16, op0=ALU.max, op1=ALU.mult,
            )
            nc.vector.tensor_scalar(
                out=Ei[:, C0 + ACT_PRE:C0 + C1], in0=m1a[:],
                scalar1=B16P, scalar2=None, op0=ALU.add,
            )
            nc.vector.reduce_sum(
                st[:, w * 5 + 2:w * 5 + 3], E[:, C0 + ACT_PRE:C0 + C1],
                axis=AX.X,
            )
            # c2 (T0): window cols [3072:4608)
            for k in range(3):
                a = k * 512
                nc.tensor.matmul(
                    pT0[:, a:a + 512], lhsT=lhs0,
                    rhs=xt2_sb[0:64, base + C0 + C1 + a:base + C0 + C1 + a + 512],
                    start=True, stop=True,
                )
            nc.scalar.activation(
                E[:, C0 + C1:C0 + C1 + C2], pT0[:], AF.Exp, bias=nshift[:],
                accum_out=st[:, w * 5 + 3:w * 5 + 4],
            )
            # c3 (T8): window cols [4608:5120)
            nc.tensor.matmul(
                pT8[:, 0:C3], lhsT=lhs8,
                rhs=xt2_sb[64:128, base + C0 + C1 + C2:base + WCOLS],
                start=True, stop=True,
            )
            m1b = m1p.tile([P, C3], f32, tag="m1b", name="m1b")
            nc.vector.tensor_scalar(
                out=m1b[:], in0=pT8[:, 0:C3],
                scalar1=CLAMP_LO, scalar2=A16, op0=ALU.max, op1=ALU.mult,
            )
            nc.vector.tensor_scalar(
                out=Ei[:, C0 + C1 + C2:WCOLS], in0=m1b[:],
                scalar1=B16P, scalar2=None, op0=ALU.add,
            )
            nc.vector.reduce_sum(
                st[:, w * 5 + 4:w * 5 + 5], E[:, C0 + C1 + C2:WCOLS],
                axis=AX.X,
            )
            prevE = E
            # possum dot: x_i . G_i for this block
            pm = pmp.tile([P, C], f32, tag="pm", name="pm")
            nc.vector.tensor_mul(
                pm[:], xrows_sb[:, w * C:(w + 1) * C],
                g_sb[:, w * C:(w + 1) * C],
            )
            nc.vector.reduce_sum(st[:, 40 + w:41 + w], pm[:], axis=AX.X)

        colsum_strips(NBLK - 1, prevE)

        # CE denominators: one exp over all logits, then per-block reduces
        cescr = epool.tile([P, NBLK * C], f32, tag="cescr", name="cescr")
        nc.scalar.activation(cescr[:], xrows_sb[:], AF.Exp)
        for w in range(NBLK):
            nc.vector.reduce_sum(
                st[:, 48 + w:49 + w], cescr[:, w * C:(w + 1) * C], axis=AX.X,
            )

        nc.sync.dma_start(stats_d.ap(), st[:])
        csa_sb = stats.tile([P, 512], f32)
        nc.vector.tensor_copy(csa_sb[:], csA[:])
        nc.sync.dma_start(csa_d.ap(), csa_sb[:])
        csb_sb = stats.tile([P, 512], f32)
        nc.vector.tensor_copy(csb_sb[:], csB[:])
        nc.sync.dma_start(csb_d.ap(), csb_sb[:])

    with tile.TileContext(nc) as tc, ExitStack() as ctx:
        emit(tc, ctx)

    nc.compile()
    return nc


def _get_nc(**kw):
    key = repr(sorted(kw.items()))
    if key not in _CACHE:
        _CACHE[key] = _build(**kw)
    return _CACHE[key]


def _prep(X, y):
    import ml_dtypes

    bf = ml_dtypes.bfloat16
    X = np.ascontiguousarray(np.asarray(X, dtype=np.float32))
    y = np.asarray(y).astype(np.int64).ravel()
    assert X.shape == (N, C) and y.shape == (N,)

    xs = (X.T / np.float32(np.sqrt(TAU))).astype(np.float32)  # [C, N]

    # class sums and per-row gathers (host, O(N*C))
    S = np.zeros((C, C), np.float64)
    np.add.at(S, y, X.astype(np.float64))
    G = S[y].astype(np.float32)                                # [N, C]
    cnt = np.bincount(y, minlength=C)[y].astype(np.float64)    # incl self
    nrm = (X.astype(np.float64) ** 2).sum(1)
    logit = X[np.arange(N), y].astype(np.float64)

    # ones-at-s window for colsum packing: col 47 is all-ones
    zwin = np.zeros((P, 184), np.float32)
    zwin[:, 47] = 1.0

    # diag-kill quadrants (64-contraction)
    dkit = np.zeros((64, 4 * P), np.float32)
    e64 = np.eye(64, dtype=np.float32)
    dkit[:, 0:64] = e64                  # lhsT TL: col p = e_p (p<64)
    dkit[:, P:P + 64] = -1e4 * e64       # rhs TL
    dkit[:, 2 * P + 64:3 * P] = e64      # lhsT BR: col p = e_{p-64} (p>=64)
    dkit[:, 3 * P + 64:4 * P] = -1e4 * e64
    zwin_bf = zwin.astype(bf)
    dkit_bf = dkit.astype(bf)

    in_maps = []
    for r in range(NCORES):
        rows = slice(r * RPC, (r + 1) * RPC)
        xl = np.roll(xs, -r * RPC, axis=1)[:, :XTW]            # [64, XTW]
        xt2 = np.concatenate([xl, xl], axis=0).astype(bf)      # [128, XTW]
        xr = X[rows].reshape(NBLK, P, C).transpose(1, 0, 2).reshape(P, NBLK * C)
        gr = G[rows].reshape(NBLK, P, C).transpose(1, 0, 2).reshape(P, NBLK * C)
        in_maps.append({
            "xt2": np.ascontiguousarray(xt2),
            "xrows": np.ascontiguousarray(xr.astype(bf)),
            "g": np.ascontiguousarray(gr.astype(bf)),
            "zwin": zwin_bf,
            "dkit": dkit_bf,
        })
    host = {"cnt": cnt, "nrm": nrm, "logit": logit}
    return in_maps, host


def _combine(results, host):
    se = np.zeros(N, np.float64)
    p_dot = np.zeros(N, np.float64)
    cesum = np.zeros(N, np.float64)
    idx = np.arange(512)
    for r, core_out in enumerate(results):
        st = core_out["stats"].astype(np.float64)
        cs = (core_out["csa"].astype(np.float64)
              + core_out["csb"].astype(np.float64))
        for w in range(NBLK):
            rows = slice((8 * r + w) * P, (8 * r + w) * P + P)
            se[rows] += st[:, w * 5:w * 5 + 5].sum(axis=1)
            p_dot[rows] = st[:, 40 + w]
            cesum[rows] = st[:, 48 + w]
            for t in range(STRIPS):
                gbase = r * RPC + w * P + P + 512 * t
                gi = (gbase + idx) % N
                se[gi] += cs[STRIPS * w + t]
    lse = np.log(se) + SHIFT
    possum = (p_dot - host["nrm"]) / TAU
    npos = host["cnt"] - 1.0
    per_i = np.where(
        npos > 0, lse - possum / np.maximum(npos, 1.0), 0.0
    )
    sc = per_i.sum()
    ce = (np.log(cesum) - host["logit"]).mean()
    return np.float32((1.0 - LMBD) * ce + LMBD * sc)


def run(input, target, trace=False, **build_kw):
    """Run the device kernel; returns (loss_scalar, BassKernelResults)."""
    from concourse.bass_utils import run_bass_kernel_spmd

    nc = _get_nc(**build_kw)
    in_maps, host = _prep(input, target)
    res = run_bass_kernel_spmd(
        nc, in_maps, core_ids=list(range(NCORES)), trace=trace
    )
    loss = _combine(res.results, host)
    return loss, res


def kernel(input, target):
    loss, _ = run(input, target, trace=False)
    return loss


# revision 9
# speedup vs baseline: 1.6309x; 1.1321x over previous
"""Fused CE + supervised-contrastive loss on 8 Trainium2 NeuronCores.

Math (reference semantics):
  ce   = -mean_i log_softmax(input)[i, y_i]
  sim  = (X @ X.T) / tau, diag excluded
  lse_i = logsumexp_{k!=i} sim[i,k]
  possum_i = (x_i . S_{y_i} - ||x_i||^2)/tau,  S_c = sum_{y_k=c} x_k
  per_i = lse_i - possum_i/n_pos_i (0 if n_pos_i == 0)
  loss = (1-lmbd)*ce + lmbd * sum_i per_i

Strategy (v2, symmetric-pair + tile-parallel PE):
  The only O(N^2) work is the exp row-sums se_i = sum_k exp(sim_ik - SHIFT).
  Each core owns 8 contiguous 128-row blocks. Row-block I processes a
  wrap-around window of W=40 column blocks j=0..39:
    - j=0: diagonal block, self-sim killed in PSUM by two 64-contraction
      "eye" accumulate-matmuls (quadrant trick keeps 64x128 tile mode).
    - j=1..24 ("symmetric"): exp'd once; row-sums via ACT/DVE accumulate,
      column-sums (the mirrored pairs' row-sums) via ones-at-s packed
      matmuls: lhsT has its ones in column s, so PSUM row s accumulates the
      strip's colsum while other rows get +0 -- 48 strips pack into one
      [128,512] PSUM tile per contraction half, drained once at the end.
    - j=25..39 ("both-directions"): distances 25..32 are computed from both
      sides (j=d and j=64-d), so no colsum is needed.
  Every matmul is 64-contraction in 64x128 tile mode: T0 instructions use
  SBUF partitions 0-63, T8 partitions 64-127, and the two tiles execute
  CONCURRENTLY (measured 2x; mode switches wedge the device, so the kernel
  never leaves 64x128 mode).  X^T/sqrt(tau) is duplicated into both
  partition halves so any chunk can go to either tile.

  exp work is split ACT (exact spline exp, ~1.2ns/col incl. overhead) vs
  DVE (Schraudolph bf16 fast-exp: clamp+scale, +bias -> int16 bitcast,
  reduce; ~2.9ns/col) to run both engines in parallel.  DVE terms carry a
  calibrated ~0.2% bias -> ~1e-5 on the final loss (tolerance 2e-2).

  Host does O(N*C) prep/post: class sums S -> G=S[y] upload, and the final
  per-row assembly (ln(se), possum, CE combine) in float64.
"""

import numpy as np

N, C = 8192, 64
NCORES = 8
RPC = N // NCORES          # 1024 rows per core
P = 128
NBLK = RPC // P            # 8 row-blocks per core
TAU = 0.5
LMBD = 0.5
SHIFT = 100.0

SYMJ = 24                  # colsum distances 1..SYMJ
WINB = 64 - SYMJ           # 40 window blocks (j = 0..39)
WCOLS = WINB * P           # 5120 window cols
CS_COLS = SYMJ * P         # 3072 colsum cols -> 6 strips of 512
STRIPS = CS_COLS // 512    # 6
XTW = (NBLK - 1) * P + WCOLS  # 6016... (7*128 + 5120) = 6016

# chunking of the 5120-col window: [T0 1536][T8 1536][T0 1024][T8 1024]
# (T8 carries more sim since T0 also does the diag kill)
C0, C1, C2, C3 = 1536, 1536, 1024, 1024
ACT_PRE = 1024             # ACT takes c1[0:1024], DVE c1[1024:1536] + c3

# Schraudolph bf16 fast-exp constants (calibrated)
A16 = 128.0 / float(np.log(2.0))
SIGMA = 3.25
B16P = 16256.0 - SIGMA - SHIFT * A16
CLAMP_LO = SHIFT - 88.0    # 12.0

_CACHE = {}


def _build():
    from contextlib import ExitStack

    import concourse.bass as bass
    import concourse.tile as tile
    from concourse import bacc, mybir

    f32 = mybir.dt.float32
    i16 = mybir.dt.int16
    bf16 = mybir.dt.bfloat16
    AF = mybir.ActivationFunctionType
    ALU = mybir.AluOpType
    AX = mybir.AxisListType

    nc = bacc.Bacc(
        "TRN2",
        target_bir_lowering=False,
        debug=False,
        num_devices=NCORES,
    )

    xt2_d = nc.dram_tensor("xt2", [P, XTW], bf16, kind="ExternalInput")
    xrows_d = nc.dram_tensor("xrows", [P, NBLK * C], bf16, kind="ExternalInput")
    g_d = nc.dram_tensor("g", [P, NBLK * C], bf16, kind="ExternalInput")
    zwin_d = nc.dram_tensor("zwin", [P, 184], bf16, kind="ExternalInput")
    dkit_d = nc.dram_tensor("dkit", [64, 4 * P], bf16, kind="ExternalInput")
    stats_d = nc.dram_tensor("stats", [P, 56], f32, kind="ExternalOutput")
    csa_d = nc.dram_tensor("csa", [P, 512], f32, kind="ExternalOutput")
    csb_d = nc.dram_tensor("csb", [P, 512], f32, kind="ExternalOutput")

    def emit(tc, ctx):
        const = ctx.enter_context(tc.tile_pool(name="const", bufs=1))
        psum = ctx.enter_context(tc.tile_pool(name="psum", bufs=1, space="PSUM"))
        epool = ctx.enter_context(tc.tile_pool(name="epool", bufs=2))
        m1p = ctx.enter_context(tc.tile_pool(name="m1p", bufs=2))
        pmp = ctx.enter_context(tc.tile_pool(name="pmp", bufs=2))
        stats = ctx.enter_context(tc.tile_pool(name="stats", bufs=1))

        # ---- input DMAs, ordered so window 0's operands land first ----
        xt2_sb = const.tile([P, XTW], bf16)
        nc.sync.dma_start(xt2_sb[:, 0:640], xt2_d.ap()[:, 0:640])
        nc.sync.dma_start(xt2_sb[:, 640:1664], xt2_d.ap()[:, 640:1664])
        zwin_sb = const.tile([P, 184], bf16)
        nc.sync.dma_start(zwin_sb[:], zwin_d.ap())
        dkit_sb = const.tile([64, 4 * P], bf16)
        nc.sync.dma_start(dkit_sb[:], dkit_d.ap())
        xrows_sb = const.tile([P, NBLK * C], bf16)
        nc.sync.dma_start(xrows_sb[:], xrows_d.ap())
        nc.sync.dma_start(xt2_sb[:, 1664:4096], xt2_d.ap()[:, 1664:4096])
        nc.sync.dma_start(xt2_sb[:, 4096:XTW], xt2_d.ap()[:, 4096:XTW])
        g_sb = const.tile([P, NBLK * C], bf16)
        nc.sync.dma_start(g_sb[:], g_d.ap())

        # ---- persistent tiles ----
        nshift = stats.tile([P, 1], f32)
        nc.vector.memset(nshift[:], -SHIFT)
        st = stats.tile([P, 56], f32)

        pT0 = psum.tile([P, C0], f32, name="pT0")
        pT8 = psum.tile([P, C1], f32, name="pT8")
        csA = psum.tile([P, 512], f32, name="csA")
        csB = psum.tile([P, 512], f32, name="csB")

        dk = dkit_sb

        def colsum_strips(w, E):
            # strips for window w read E regions produced ~a window ago, so
            # emitting them inside window w+1 keeps the in-order PE queue
            # from stalling on the freshest exp region
            for t in range(STRIPS):
                s = STRIPS * w + t
                a = P + 512 * t
                nc.tensor.matmul(
                    csA[:], lhsT=zwin_sb[0:64, 47 - s:47 - s + P],
                    rhs=E[0:64, a:a + 512],
                    start=(s == 0), stop=(s == STRIPS * NBLK - 1),
                    skip_group_check=True,
                )
                nc.tensor.matmul(
                    csB[:], lhsT=zwin_sb[64:128, 47 - s:47 - s + P],
                    rhs=E[64:128, a:a + 512],
                    start=(s == 0), stop=(s == STRIPS * NBLK - 1),
                    skip_group_check=True,
                )

        prevE = None
        for w in range(NBLK):
            base = w * P
            lhs0 = xt2_sb[0:64, base:base + P]
            lhs8 = xt2_sb[64:128, base:base + P]
            E = epool.tile([P, WCOLS], bf16, tag="E", name="E")
            Ei = E[:].bitcast(i16)

            # c0 (T0): window cols [0:1536)
            for k in range(3):
                a = k * 512
                nc.tensor.matmul(
                    pT0[:, a:a + 512], lhsT=lhs0,
                    rhs=xt2_sb[0:64, base + a:base + a + 512],
                    start=True, stop=True,
                )
            # diagonal kill on cols [0:128) (two 64-contraction quadrants)
            nc.tensor.matmul(
                pT0[:, 0:P], lhsT=dk[:, 0:P], rhs=dk[:, P:2 * P],
                start=False, stop=True, skip_group_check=True,
            )
            nc.tensor.matmul(
                pT0[:, 0:P], lhsT=dk[:, 2 * P:3 * P], rhs=dk[:, 3 * P:4 * P],
                start=False, stop=True, skip_group_check=True,
            )
            # c1 (T8): window cols [1536:3072)
            for k in range(3):
                a = k * 512
                nc.tensor.matmul(
                    pT8[:, a:a + 512], lhsT=lhs8,
                    rhs=xt2_sb[64:128, base + C0 + a:base + C0 + a + 512],
                    start=True, stop=True,
                )
            # previous window's colsum strips (data long ready: no PE stall)
            if prevE is not None:
                colsum_strips(w - 1, prevE)
            # ACT exp c0 -> E[0:1536), rowsum to st col w*5+0
            nc.scalar.activation(
                E[:, 0:C0], pT0[:], AF.Exp, bias=nshift[:],
                accum_out=st[:, w * 5:w * 5 + 1],
            )
            # ACT exp c1 prefix [1536:2176)
            nc.scalar.activation(
                E[:, C0:C0 + ACT_PRE], pT8[:, 0:ACT_PRE], AF.Exp,
                bias=nshift[:], accum_out=st[:, w * 5 + 1:w * 5 + 2],
            )
            # DVE fast-exp c1 suffix [2176:3072)
            m1a = m1p.tile([P, C1 - ACT_PRE], f32, tag="m1a", name="m1a")
            nc.vector.tensor_scalar(
                out=m1a[:], in0=pT8[:, ACT_PRE:C1],
                scalar1=CLAMP_LO, scalar2=A16, op0=ALU.max, op1=ALU.mult,
            )
            nc.vector.tensor_scalar(
                out=Ei[:, C0 + ACT_PRE:C0 + C1], in0=m1a[:],
                scalar1=B16P, scalar2=None, op0=ALU.add,
            )
            nc.vector.reduce_sum(
                st[:, w * 5 + 2:w * 5 + 3], E[:, C0 + ACT_PRE:C0 + C1],
                axis=AX.X,
            )
            # c2 (T0): window cols [3072:4096)
            for k in range(2):
                a = k * 512
                nc.tensor.matmul(
                    pT0[:, a:a + 512], lhsT=lhs0,
                    rhs=xt2_sb[0:64, base + C0 + C1 + a:base + C0 + C1 + a + 512],
                    start=True, stop=True,
                )
            nc.scalar.activation(
                E[:, C0 + C1:C0 + C1 + C2], pT0[:, 0:C2], AF.Exp,
                bias=nshift[:], accum_out=st[:, w * 5 + 3:w * 5 + 4],
            )
            # c3 (T8): window cols [4096:5120)
            for k in range(2):
                a = k * 512
                nc.tensor.matmul(
                    pT8[:, a:a + 512], lhsT=lhs8,
                    rhs=xt2_sb[64:128, base + C0 + C1 + C2 + a:base + C0 + C1 + C2 + a + 512],
                    start=True, stop=True,
                )
            m1b = m1p.tile([P, C3], f32, tag="m1b", name="m1b")
            nc.vector.tensor_scalar(
                out=m1b[:], in0=pT8[:, 0:C3],
                scalar1=CLAMP_LO, scalar2=A16, op0=ALU.max, op1=ALU.mult,
            )
            nc.vector.tensor_scalar(
                out=Ei[:, C0 + C1 + C2:WCOLS], in0=m1b[:],
                scalar1=B16P, scalar2=None, op0=ALU.add,
            )
            nc.vector.reduce_sum(
                st[:, w * 5 + 4:w * 5 + 5], E[:, C0 + C1 + C2:WCOLS],
                axis=AX.X,
            )
            prevE = E
            # possum dot: x_i . G_i for this block
            pm = pmp.tile([P, C], f32, tag="pm", name="pm")
            nc.vector.tensor_mul(
                pm[:], xrows_sb[:, w * C:(w + 1) * C],
                g_sb[:, w * C:(w + 1) * C],
            )
            nc.vector.reduce_sum(st[:, 40 + w:41 + w], pm[:], axis=AX.X)

            if w == 1:
                # CE denominators, emitted early so they hide under the PE
                cescr = epool.tile([P, NBLK * C], f32, tag="cescr",
                                   name="cescr")
                nc.scalar.activation(cescr[:], xrows_sb[:], AF.Exp)
                for v in range(NBLK):
                    nc.vector.reduce_sum(
                        st[:, 48 + v:49 + v], cescr[:, v * C:(v + 1) * C],
                        axis=AX.X,
                    )

        colsum_strips(NBLK - 1, prevE)

        nc.sync.dma_start(stats_d.ap(), st[:])
        csa_sb = stats.tile([P, 512], f32)
        nc.vector.tensor_copy(csa_sb[:], csA[:])
        nc.sync.dma_start(csa_d.ap(), csa_sb[:])
        csb_sb = stats.tile([P, 512], f32)
        nc.vector.tensor_copy(csb_sb[:], csB[:])
        nc.sync.dma_start(csb_d.ap(), csb_sb[:])

    with tile.TileContext(nc) as tc, ExitStack() as ctx:
        emit(tc, ctx)

    nc.compile()
    return nc


def _get_nc(**kw):
    key = repr(sorted(kw.items()))
    if key not in _CACHE:
        _CACHE[key] = _build(**kw)
    return _CACHE[key]


def _prep(X, y):
    import ml_dtypes

    bf = ml_dtypes.bfloat16
    X = np.ascontiguousarray(np.asarray(X, dtype=np.float32))
    y = np.asarray(y).astype(np.int64).ravel()
    assert X.shape == (N, C) and y.shape == (N,)

    xs = (X.T / np.float32(np.sqrt(TAU))).astype(np.float32)  # [C, N]

    # class sums and per-row gathers (host, O(N*C))
    S = np.zeros((C, C), np.float64)
    np.add.at(S, y, X.astype(np.float64))
    G = S[y].astype(np.float32)                                # [N, C]
    cnt = np.bincount(y, minlength=C)[y].astype(np.float64)    # incl self
    nrm = (X.astype(np.float64) ** 2).sum(1)
    logit = X[np.arange(N), y].astype(np.float64)

    # ones-at-s window for colsum packing: col 47 is all-ones
    zwin = np.zeros((P, 184), np.float32)
    zwin[:, 47] = 1.0

    # diag-kill quadrants (64-contraction)
    dkit = np.zeros((64, 4 * P), np.float32)
    e64 = np.eye(64, dtype=np.float32)
    dkit[:, 0:64] = e64                  # lhsT TL: col p = e_p (p<64)
    dkit[:, P:P + 64] = -1e4 * e64       # rhs TL
    dkit[:, 2 * P + 64:3 * P] = e64      # lhsT BR: col p = e_{p-64} (p>=64)
    dkit[:, 3 * P + 64:4 * P] = -1e4 * e64
    zwin_bf = zwin.astype(bf)
    dkit_bf = dkit.astype(bf)

    in_maps = []
    for r in range(NCORES):
        rows = slice(r * RPC, (r + 1) * RPC)
        xl = np.roll(xs, -r * RPC, axis=1)[:, :XTW]            # [64, XTW]
        xt2 = np.concatenate([xl, xl], axis=0).astype(bf)      # [128, XTW]
        xr = X[rows].reshape(NBLK, P, C).transpose(1, 0, 2).reshape(P, NBLK * C)
        gr = G[rows].reshape(NBLK, P, C).transpose(1, 0, 2).reshape(P, NBLK * C)
        in_maps.append({
            "xt2": np.ascontiguousarray(xt2),
            "xrows": np.ascontiguousarray(xr.astype(bf)),
            "g": np.ascontiguousarray(gr.astype(bf)),
            "zwin": zwin_bf,
            "dkit": dkit_bf,
        })
    host = {"cnt": cnt, "nrm": nrm, "logit": logit}
    return in_maps, host


def _combine(results, host):
    se = np.zeros(N, np.float64)
    p_dot = np.zeros(N, np.float64)
    cesum = np.zeros(N, np.float64)
    idx = np.arange(512)
    for r, core_out in enumerate(results):
        st = core_out["stats"].astype(np.float64)
        cs = (core_out["csa"].astype(np.float64)
              + core_out["csb"].astype(np.float64))
        for w in range(NBLK):
            rows = slice((8 * r + w) * P, (8 * r + w) * P + P)
            se[rows] += st[:, w * 5:w * 5 + 5].sum(axis=1)
            p_dot[rows] = st[:, 40 + w]
            cesum[rows] = st[:, 48 + w]
            for t in range(STRIPS):
                gbase = r * RPC + w * P + P + 512 * t
                gi = (gbase + idx) % N
                se[gi] += cs[STRIPS * w + t]
    lse = np.log(se) + SHIFT
    possum = (p_dot - host["nrm"]) / TAU
    npos = host["cnt"] - 1.0
    per_i = np.where(
        npos > 0, lse - possum / np.maximum(npos, 1.0), 0.0
    )
    sc = per_i.sum()
    ce = (np.log(cesum) - host["logit"]).mean()
    return np.float32((1.0 - LMBD) * ce + LMBD * sc)


def run(input, target, trace=False, **build_kw):
    """Run the device kernel; returns (loss_scalar, BassKernelResults)."""
    from concourse.bass_utils import run_bass_kernel_spmd

    nc = _get_nc(**build_kw)
    in_maps, host = _prep(input, target)
    res = run_bass_kernel_spmd(
        nc, in_maps, core_ids=list(range(NCORES)), trace=trace
    )
    loss = _combine(res.results, host)
    return loss, res


def kernel(input, target):
    loss, _ = run(input, target, trace=False)
    return loss


# revision 10
# speedup vs baseline: 1.6991x; 1.0418x over previous
"""Fused CE + supervised-contrastive loss on 8 Trainium2 NeuronCores.

Math (reference semantics):
  ce   = -mean_i log_softmax(input)[i, y_i]
  sim  = (X @ X.T) / tau, diag excluded
  lse_i = logsumexp_{k!=i} sim[i,k]
  possum_i = (x_i . S_{y_i} - ||x_i||^2)/tau,  S_c = sum_{y_k=c} x_k
  per_i = lse_i - possum_i/n_pos_i (0 if n_pos_i == 0)
  loss = (1-lmbd)*ce + lmbd * sum_i per_i

Strategy (v2, symmetric-pair + tile-parallel PE):
  The only O(N^2) work is the exp row-sums se_i = sum_k exp(sim_ik - SHIFT).
  Each core owns 8 contiguous 128-row blocks. Row-block I processes a
  wrap-around window of W=40 column blocks j=0..39:
    - j=0: diagonal block, self-sim killed in PSUM by two 64-contraction
      "eye" accumulate-matmuls (quadrant trick keeps 64x128 tile mode).
    - j=1..24 ("symmetric"): exp'd once; row-sums via ACT/DVE accumulate,
      column-sums (the mirrored pairs' row-sums) via ones-at-s packed
      matmuls: lhsT has its ones in column s, so PSUM row s accumulates the
      strip's colsum while other rows get +0 -- 48 strips pack into one
      [128,512] PSUM tile per contraction half, drained once at the end.
    - j=25..39 ("both-directions"): distances 25..32 are computed from both
      sides (j=d and j=64-d), so no colsum is needed.
  Every matmul is 64-contraction in 64x128 tile mode: T0 instructions use
  SBUF partitions 0-63, T8 partitions 64-127, and the two tiles execute
  CONCURRENTLY (measured 2x; mode switches wedge the device, so the kernel
  never leaves 64x128 mode).  X^T/sqrt(tau) is duplicated into both
  partition halves so any chunk can go to either tile.

  exp work is split ACT (exact spline exp, ~1.2ns/col incl. overhead) vs
  DVE (Schraudolph bf16 fast-exp: clamp+scale, +bias -> int16 bitcast,
  reduce; ~2.9ns/col) to run both engines in parallel.  DVE terms carry a
  calibrated ~0.2% bias -> ~1e-5 on the final loss (tolerance 2e-2).

  Host does O(N*C) prep/post: class sums S -> G=S[y] upload, and the final
  per-row assembly (ln(se), possum, CE combine) in float64.
"""

import numpy as np

N, C = 8192, 64
NCORES = 8
RPC = N // NCORES          # 1024 rows per core
P = 128
NBLK = RPC // P            # 8 row-blocks per core
TAU = 0.5
LMBD = 0.5
SHIFT = 100.0

SYMJ = 24                  # colsum distances 1..SYMJ
WINB = 64 - SYMJ           # 40 window blocks (j = 0..39)
WCOLS = WINB * P           # 5120 window cols
CS_COLS = SYMJ * P         # 3072 colsum cols -> 6 strips of 512
STRIPS = CS_COLS // 512    # 6
XTW = (NBLK - 1) * P + WCOLS  # 6016... (7*128 + 5120) = 6016

# chunking of the 5120-col window: [T0 1536][T8 1536][T0 1024][T8 1024]
# (T8 carries more sim since T0 also does the diag kill)
C0, C1, C2, C3 = 1536, 1536, 1024, 1024
ACT_PRE = 1024             # ACT takes c1[0:1024], DVE c1[1024:1536] + c3

# Schraudolph bf16 fast-exp constants (calibrated)
A16 = 128.0 / float(np.log(2.0))
SIGMA = 3.25
B16P = 16256.0 - SIGMA - SHIFT * A16
CLAMP_LO = SHIFT - 88.0    # 12.0

_CACHE = {}


def _build():
    from contextlib import ExitStack

    import concourse.bass as bass
    import concourse.tile as tile
    from concourse import bacc, mybir

    f32 = mybir.dt.float32
    i16 = mybir.dt.int16
    bf16 = mybir.dt.bfloat16
    AF = mybir.ActivationFunctionType
    ALU = mybir.AluOpType
    AX = mybir.AxisListType

    nc = bacc.Bacc(
        "TRN2",
        target_bir_lowering=False,
        debug=False,
        num_devices=NCORES,
    )

    xt2_d = nc.dram_tensor("xt2", [P, XTW], bf16, kind="ExternalInput")
    xrows_d = nc.dram_tensor("xrows", [P, NBLK * C], bf16, kind="ExternalInput")
    g_d = nc.dram_tensor("g", [P, NBLK * C], bf16, kind="ExternalInput")
    zwin_d = nc.dram_tensor("zwin", [P, 184], bf16, kind="ExternalInput")
    dkit_d = nc.dram_tensor("dkit", [64, 4 * P], bf16, kind="ExternalInput")
    stats_d = nc.dram_tensor("stats", [P, 56], f32, kind="ExternalOutput")
    csa_d = nc.dram_tensor("csa", [P, 512], f32, kind="ExternalOutput")
    csb_d = nc.dram_tensor("csb", [P, 512], f32, kind="ExternalOutput")

    def emit(tc, ctx):
        const = ctx.enter_context(tc.tile_pool(name="const", bufs=1))
        psum = ctx.enter_context(tc.tile_pool(name="psum", bufs=1, space="PSUM"))
        epool = ctx.enter_context(tc.tile_pool(name="epool", bufs=2))
        m1p = ctx.enter_context(tc.tile_pool(name="m1p", bufs=2))
        pmp = ctx.enter_context(tc.tile_pool(name="pmp", bufs=2))
        stats = ctx.enter_context(tc.tile_pool(name="stats", bufs=1))

        # ---- input DMAs, ordered so window 0's operands land first ----
        xt2_sb = const.tile([P, XTW], bf16)
        nc.sync.dma_start(xt2_sb[:, 0:1664], xt2_d.ap()[:, 0:1664])
        dkit_sb = const.tile([64, 4 * P], bf16)
        nc.sync.dma_start(dkit_sb[:], dkit_d.ap())
        nc.sync.dma_start(xt2_sb[:, 1664:4096], xt2_d.ap()[:, 1664:4096])
        zwin_sb = const.tile([P, 184], bf16)
        nc.sync.dma_start(zwin_sb[:], zwin_d.ap())
        nc.sync.dma_start(xt2_sb[:, 4096:XTW], xt2_d.ap()[:, 4096:XTW])
        xrows_sb = const.tile([P, NBLK * C], bf16)
        nc.sync.dma_start(xrows_sb[:], xrows_d.ap())
        g_sb = const.tile([P, NBLK * C], bf16)
        nc.sync.dma_start(g_sb[:], g_d.ap())

        # ---- persistent tiles ----
        nshift = stats.tile([P, 1], f32)
        nc.vector.memset(nshift[:], -SHIFT)
        st = stats.tile([P, 56], f32)

        pT0 = psum.tile([P, C0], f32, name="pT0")
        pT8 = psum.tile([P, C1], f32, name="pT8")
        csA = psum.tile([P, 512], f32, name="csA")
        csB = psum.tile([P, 512], f32, name="csB")

        dk = dkit_sb

        def colsum_strips(w, E):
            # strips for window w read E regions produced ~a window ago, so
            # emitting them inside window w+1 keeps the in-order PE queue
            # from stalling on the freshest exp region
            for t in range(STRIPS):
                s = STRIPS * w + t
                a = P + 512 * t
                nc.tensor.matmul(
                    csA[:], lhsT=zwin_sb[0:64, 47 - s:47 - s + P],
                    rhs=E[0:64, a:a + 512],
                    start=(s == 0), stop=(s == STRIPS * NBLK - 1),
                    skip_group_check=True,
                )
                nc.tensor.matmul(
                    csB[:], lhsT=zwin_sb[64:128, 47 - s:47 - s + P],
                    rhs=E[64:128, a:a + 512],
                    start=(s == 0), stop=(s == STRIPS * NBLK - 1),
                    skip_group_check=True,
                )

        prevE = None
        for w in range(NBLK):
            base = w * P
            lhs0 = xt2_sb[0:64, base:base + P]
            lhs8 = xt2_sb[64:128, base:base + P]
            E = epool.tile([P, WCOLS], bf16, tag="E", name="E")
            Ei = E[:].bitcast(i16)

            # c0 (T0): window cols [0:1536)
            for k in range(3):
                a = k * 512
                nc.tensor.matmul(
                    pT0[:, a:a + 512], lhsT=lhs0,
                    rhs=xt2_sb[0:64, base + a:base + a + 512],
                    start=True, stop=True,
                )
            # diagonal kill on cols [0:128) (two 64-contraction quadrants)
            nc.tensor.matmul(
                pT0[:, 0:P], lhsT=dk[:, 0:P], rhs=dk[:, P:2 * P],
                start=False, stop=True, skip_group_check=True,
            )
            nc.tensor.matmul(
                pT0[:, 0:P], lhsT=dk[:, 2 * P:3 * P], rhs=dk[:, 3 * P:4 * P],
                start=False, stop=True, skip_group_check=True,
            )
            # c1 (T8): window cols [1536:3072)
            for k in range(3):
                a = k * 512
                nc.tensor.matmul(
                    pT8[:, a:a + 512], lhsT=lhs8,
                    rhs=xt2_sb[64:128, base + C0 + a:base + C0 + a + 512],
                    start=True, stop=True,
                )
            # previous window's colsum strips (data long ready: no PE stall)
            if prevE is not None:
                colsum_strips(w - 1, prevE)
            # ACT exp c0 -> E[0:1536), rowsum to st col w*5+0
            nc.scalar.activation(
                E[:, 0:C0], pT0[:], AF.Exp, bias=nshift[:],
                accum_out=st[:, w * 5:w * 5 + 1],
            )
            # ACT exp c1 prefix [1536:2176)
            nc.scalar.activation(
                E[:, C0:C0 + ACT_PRE], pT8[:, 0:ACT_PRE], AF.Exp,
                bias=nshift[:], accum_out=st[:, w * 5 + 1:w * 5 + 2],
            )
            # DVE fast-exp c1 suffix [2176:3072)
            m1a = m1p.tile([P, C1 - ACT_PRE], f32, tag="m1a", name="m1a")
            nc.vector.tensor_scalar(
                out=m1a[:], in0=pT8[:, ACT_PRE:C1],
                scalar1=CLAMP_LO, scalar2=A16, op0=ALU.max, op1=ALU.mult,
            )
            nc.vector.tensor_scalar(
                out=Ei[:, C0 + ACT_PRE:C0 + C1], in0=m1a[:],
                scalar1=B16P, scalar2=None, op0=ALU.add,
            )
            nc.vector.reduce_sum(
                st[:, w * 5 + 2:w * 5 + 3], E[:, C0 + ACT_PRE:C0 + C1],
                axis=AX.X,
            )
            # c2 (T0): window cols [3072:4096)
            for k in range(2):
                a = k * 512
                nc.tensor.matmul(
                    pT0[:, a:a + 512], lhsT=lhs0,
                    rhs=xt2_sb[0:64, base + C0 + C1 + a:base + C0 + C1 + a + 512],
                    start=True, stop=True,
                )
            nc.scalar.activation(
                E[:, C0 + C1:C0 + C1 + C2], pT0[:, 0:C2], AF.Exp,
                bias=nshift[:], accum_out=st[:, w * 5 + 3:w * 5 + 4],
            )
            # c3 (T8): window cols [4096:5120)
            for k in range(2):
                a = k * 512
                nc.tensor.matmul(
                    pT8[:, a:a + 512], lhsT=lhs8,
                    rhs=xt2_sb[64:128, base + C0 + C1 + C2 + a:base + C0 + C1 + C2 + a + 512],
                    start=True, stop=True,
                )
            if w < NBLK - 1:
                m1b = m1p.tile([P, C3], f32, tag="m1b", name="m1b")
                nc.vector.tensor_scalar(
                    out=m1b[:], in0=pT8[:, 0:C3],
                    scalar1=CLAMP_LO, scalar2=A16, op0=ALU.max, op1=ALU.mult,
                )
                nc.vector.tensor_scalar(
                    out=Ei[:, C0 + C1 + C2:WCOLS], in0=m1b[:],
                    scalar1=B16P, scalar2=None, op0=ALU.add,
                )
                nc.vector.reduce_sum(
                    st[:, w * 5 + 4:w * 5 + 5], E[:, C0 + C1 + C2:WCOLS],
                    axis=AX.X,
                )
            else:
                nc.scalar.activation(
                    E[:, C0 + C1 + C2:WCOLS], pT8[:, 0:C3], AF.Exp,
                    bias=nshift[:], accum_out=st[:, w * 5 + 4:w * 5 + 5],
                )
            prevE = E
            # possum dot: x_i . G_i for this block
            pm = pmp.tile([P, C], f32, tag="pm", name="pm")
            nc.vector.tensor_mul(
                pm[:], xrows_sb[:, w * C:(w + 1) * C],
                g_sb[:, w * C:(w + 1) * C],
            )
            nc.vector.reduce_sum(st[:, 40 + w:41 + w], pm[:], axis=AX.X)

            if w == 1:
                # CE denominators, emitted early so they hide under the PE
                cescr = epool.tile([P, NBLK * C], f32, tag="cescr",
                                   name="cescr")
                nc.scalar.activation(cescr[:], xrows_sb[:], AF.Exp)
                for v in range(NBLK):
                    nc.vector.reduce_sum(
                        st[:, 48 + v:49 + v], cescr[:, v * C:(v + 1) * C],
                        axis=AX.X,
                    )

        colsum_strips(NBLK - 1, prevE)

        nc.sync.dma_start(stats_d.ap(), st[:])
        csa_sb = stats.tile([P, 512], f32)
        nc.vector.tensor_copy(csa_sb[:], csA[:])
        nc.sync.dma_start(csa_d.ap(), csa_sb[:])
        csb_sb = stats.tile([P, 512], f32)
        nc.vector.tensor_copy(csb_sb[:], csB[:])
        nc.sync.dma_start(csb_d.ap(), csb_sb[:])

    with tile.TileContext(nc) as tc, ExitStack() as ctx:
        emit(tc, ctx)

    nc.compile()
    return nc


def _get_nc(**kw):
    key = repr(sorted(kw.items()))
    if key not in _CACHE:
        _CACHE[key] = _build(**kw)
    return _CACHE[key]


def _prep(X, y):
    import ml_dtypes

    bf = ml_dtypes.bfloat16
    X = np.ascontiguousarray(np.asarray(X, dtype=np.float32))
    y = np.asarray(y).astype(np.int64).ravel()
    assert X.shape == (N, C) and y.shape == (N,)

    xs = (X.T / np.float32(np.sqrt(TAU))).astype(np.float32)  # [C, N]

    # class sums and per-row gathers (host, O(N*C))
    S = np.zeros((C, C), np.float64)
    np.add.at(S, y, X.astype(np.float64))
    G = S[y].astype(np.float32)                                # [N, C]
    cnt = np.bincount(y, minlength=C)[y].astype(np.float64)    # incl self
    nrm = (X.astype(np.float64) ** 2).sum(1)
    logit = X[np.arange(N), y].astype(np.float64)

    # ones-at-s window for colsum packing: col 47 is all-ones
    zwin = np.zeros((P, 184), np.float32)
    zwin[:, 47] = 1.0

    # diag-kill quadrants (64-contraction)
    dkit = np.zeros((64, 4 * P), np.float32)
    e64 = np.eye(64, dtype=np.float32)
    dkit[:, 0:64] = e64                  # lhsT TL: col p = e_p (p<64)
    dkit[:, P:P + 64] = -1e4 * e64       # rhs TL
    dkit[:, 2 * P + 64:3 * P] = e64      # lhsT BR: col p = e_{p-64} (p>=64)
    dkit[:, 3 * P + 64:4 * P] = -1e4 * e64
    zwin_bf = zwin.astype(bf)
    dkit_bf = dkit.astype(bf)

    in_maps = []
    for r in range(NCORES):
        rows = slice(r * RPC, (r + 1) * RPC)
        xl = np.roll(xs, -r * RPC, axis=1)[:, :XTW]            # [64, XTW]
        xt2 = np.concatenate([xl, xl], axis=0).astype(bf)      # [128, XTW]
        xr = X[rows].reshape(NBLK, P, C).transpose(1, 0, 2).reshape(P, NBLK * C)
        gr = G[rows].reshape(NBLK, P, C).transpose(1, 0, 2).reshape(P, NBLK * C)
        in_maps.append({
            "xt2": np.ascontiguousarray(xt2),
            "xrows": np.ascontiguousarray(xr.astype(bf)),
            "g": np.ascontiguousarray(gr.astype(bf)),
            "zwin": zwin_bf,
            "dkit": dkit_bf,
        })
    host = {"cnt": cnt, "nrm": nrm, "logit": logit}
    return in_maps, host


def _combine(results, host):
    se = np.zeros(N, np.float64)
    p_dot = np.zeros(N, np.float64)
    cesum = np.zeros(N, np.float64)
    idx = np.arange(512)
    for r, core_out in enumerate(results):
        st = core_out["stats"].astype(np.float64)
        cs = (core_out["csa"].astype(np.float64)
              + core_out["csb"].astype(np.float64))
        for w in range(NBLK):
            rows = slice((8 * r + w) * P, (8 * r + w) * P + P)
            se[rows] += st[:, w * 5:w * 5 + 5].sum(axis=1)
            p_dot[rows] = st[:, 40 + w]
            cesum[rows] = st[:, 48 + w]
            for t in range(STRIPS):
                gbase = r * RPC + w * P + P + 512 * t
                gi = (gbase + idx) % N
                se[gi] += cs[STRIPS * w + t]
    lse = np.log(se) + SHIFT
    possum = (p_dot - host["nrm"]) / TAU
    npos = host["cnt"] - 1.0
    per_i = np.where(
        npos > 0, lse - possum / np.maximum(npos, 1.0), 0.0
    )
    sc = per_i.sum()
    ce = (np.log(cesum) - host["logit"]).mean()
    return np.float32((1.0 - LMBD) * ce + LMBD * sc)


def run(input, target, trace=False, **build_kw):
    """Run the device kernel; returns (loss_scalar, BassKernelResults)."""
    from concourse.bass_utils import run_bass_kernel_spmd

    nc = _get_nc(**build_kw)
    in_maps, host = _prep(input, target)
    res = run_bass_kernel_spmd(
        nc, in_maps, core_ids=list(range(NCORES)), trace=trace
    )
    loss = _combine(res.results, host)
    return loss, res


def kernel(input, target):
    loss, _ = run(input, target, trace=False)
    return loss
